# revision 7
# baseline (speedup 1.0000x reference)
"""Trainium2 Bass kernel for 16-head causal MHA (B=2, T=2048, C=1024, H=16, D=64).

Sharding: 8 cores = 2 batch groups x 4 head groups (4 heads each).
Each core computes, for its batch b and heads hg*4..hg*4+3:
  Q^T,K^T = (W^T x^T) projections kept transposed [dims, tokens]
  V       = projection transposed back to [tokens, dims] via PE transpose
  S^T     = K Q^T per (ts-tile, tq-chunk), causal-masked, exp'd (scale folded
            into Wq on host), denominator via ones-column appended to V
  O^T     = V_aug^T P^T accumulated over ts tiles, normalized by 1/denom
  Y_part  = O^T.T @ Wo_slice^T  (partial out-projection, [2048, 1024])
Host sums the 4 head-group partials per batch and adds bo.
"""

import sys

sys.path.insert(0, "/opt/trn_rl_repo")

import numpy as np

import concourse.bass as bass
from concourse import bacc
import concourse.mybir as mybir
from concourse.tile import TileContext
from concourse.bass_utils import run_bass_kernel_spmd
from concourse.masks import make_identity

F32 = mybir.dt.float32
F32R = mybir.dt.float32r
EXP = mybir.ActivationFunctionType.Exp

B, T, C, H, D = 2, 2048, 1024, 16, 64
NHPC = 4          # heads per core
DH = NHPC * D     # 256 head dims per core
P = 128           # partitions
CH = 512          # token chunk (matmul moving dim)
NCHUNK = T // CH  # 4
NTT = T // P      # 16 token tiles
NCT = C // P      # 8 contraction tiles over C
NEG = -1.0e10


def build_nc():
    nc = bacc.Bacc()
    xT_d = nc.declare_dram_parameter("xT", [C, T], F32R, isOutput=False)
    wqkv_d = nc.declare_dram_parameter("Wqkv", [C, 3 * DH], F32R, isOutput=False)
    wot_d = nc.declare_dram_parameter("WoT", [DH, C], F32R, isOutput=False)
    y_d = nc.declare_dram_parameter("Y", [T, C], F32, isOutput=True)

    xT = xT_d[:, :]
    wqkv = wqkv_d[:, :]
    wot = wot_d[:, :]
    y = y_d[:, :]

    with TileContext(nc) as tc:
        with (
            tc.tile_pool(name="const", bufs=1) as const,
            tc.tile_pool(name="persist", bufs=1) as persist,
        ):
            # ---- constants ----
            identity = const.tile([P, P], F32)
            make_identity(nc, identity)
            ones_f32 = const.tile([P, 1], F32)
            nc.gpsimd.memset(ones_f32[:], 1.0)
            ones_row_f32 = const.tile([1, D], F32)
            nc.gpsimd.memset(ones_row_f32[:], 1.0)
            ones_col = const.tile([1, D], F32R)
            nc.vector.tensor_copy(ones_col[:], ones_row_f32[:])
            # causal masks for diagonal tiles, k = ts_tile_offset within chunk.
            # mask[k][r, (half, j)] = 0 if r <= j - 128k else -1e10 (both halves)
            masks = []
            for k in range(4):
                mk = const.tile([P, 2, CH], F32, name=f"mask{k}")
                nc.gpsimd.memset(mk[:], 0.0)
                nc.gpsimd.affine_select(
                    out=mk[:],
                    in_=mk[:],
                    compare_op=mybir.AluOpType.is_ge,
                    fill=NEG,
                    base=-P * k,
                    # keep where (-1)*part + 0*half + 1*j - 128k >= 0
                    pattern=[[0, 2], [1, CH]],
                    channel_multiplier=-1,
                )
                masks.append(mk)

            # ---- persistent tensors ----
            wq_t = []
            for c in range(NCT):
                wt = persist.tile([P, 3 * DH], F32R, name=f"wqkv{c}")
                nc.sync.dma_start(wt[:], wqkv[c * P : (c + 1) * P, :])
                wq_t.append(wt)
            wot_t = []
            for k in range(2):
                wo = persist.tile([P, C], F32R, name=f"wot{k}")
                nc.sync.dma_start(wo[:], wot[k * P : (k + 1) * P, :])
                wot_t.append(wo)

            # Q^T/K^T [dims, tokens]; pair p holds heads (2p, 2p+1)
            qt_t = [persist.tile([P, T], F32R, name=f"qt{p}") for p in range(2)]
            kt_t = [persist.tile([P, T], F32R, name=f"kt{p}") for p in range(2)]
            # V augmented with a ones column per head: [tokens, 4*65]
            vaug_t = [persist.tile([P, NHPC * (D + 1)], F32R, name=f"vaug{t}")
                      for t in range(NTT)]
            for t in range(NTT):
                for h in range(NHPC):
                    col = h * (D + 1) + D
                    nc.vector.tensor_copy(vaug_t[t][:, col : col + 1], ones_f32[:])
            # normalized O^T [dims, tokens]
            ot_t = [persist.tile([P, T], F32R, name=f"ot{p}") for p in range(2)]

            # ================= stage 1: projections =================
            with (
                tc.tile_pool(name="xt", bufs=16) as xt_pool,
                tc.tile_pool(name="vtmp", bufs=2) as vtmp_pool,
                tc.tile_pool(name="psproj", bufs=2, space="PSUM") as ps_proj,
                tc.tile_pool(name="pstp", bufs=2, space="PSUM") as ps_tp,
            ):
                for n in range(NCHUNK):
                    csl = slice(n * CH, (n + 1) * CH)
                    xts = []
                    for c in range(NCT):
                        xtile = xt_pool.tile([P, CH], F32R, tag="xt", name=f"xt{n}_{c}")
                        nc.sync.dma_start(xtile[:], xT[c * P : (c + 1) * P, csl])
                        xts.append(xtile)
                    for m in range(6):
                        ps = ps_proj.tile([P, CH], F32, tag="ps", name=f"ps{n}_{m}")
                        for c in range(NCT):
                            nc.tensor.matmul(
                                ps[:],
                                lhsT=wq_t[c][:, m * P : (m + 1) * P].bitcast(F32R),
                                rhs=xts[c][:].bitcast(F32R),
                                start=(c == 0),
                                stop=(c == NCT - 1),
                            )
                        if m < 2:
                            nc.vector.tensor_copy(qt_t[m][:, csl], ps[:])
                        elif m < 4:
                            nc.vector.tensor_copy(kt_t[m - 2][:, csl], ps[:])
                        else:
                            # V^T [128 vdims, 512 tokens] -> transpose to V
                            vt = vtmp_pool.tile([P, CH], F32, tag="vt",
                                                name=f"vt{n}_{m}")
                            nc.vector.tensor_copy(vt[:], ps[:])
                            for hh in range(2):
                                h = 2 * (m - 4) + hh
                                for j in range(4):
                                    tp = ps_tp.tile([P, D], F32, tag="tp",
                                                    name=f"tp{n}_{m}_{hh}_{j}")
                                    nc.tensor.transpose(
                                        tp[:],
                                        vt[hh * D : (hh + 1) * D,
                                           j * P : (j + 1) * P],
                                        identity[hh * D : (hh + 1) * D,
                                                 hh * D : (hh + 1) * D],
                                    )
                                    nc.vector.tensor_copy(
                                        vaug_t[4 * n + j][:, h * (D + 1) : h * (D + 1) + D],
                                        tp[:],
                                    )

            # ================= stage 2: attention =================
            with (
                tc.tile_pool(name="pt", bufs=3) as pt_pool,
                tc.tile_pool(name="small", bufs=4) as small_pool,
                tc.tile_pool(name="psst", bufs=2, space="PSUM") as ps_st,
                tc.tile_pool(name="psot", bufs=4, space="PSUM") as ps_ot,
            ):
                for cq in range(NCHUNK):
                    qsl = slice(cq * CH, (cq + 1) * CH)
                    nts = 4 * cq + 4
                    for p in range(2):
                        ot0 = ps_ot.tile([D + 1, CH], F32, tag="ot", name=f"ot{cq}_{p}_0")
                        ot1 = ps_ot.tile([D + 1, CH], F32, tag="ot", name=f"ot{cq}_{p}_1")
                        for t in range(nts):
                            st = ps_st.tile([P, 2, CH], F32, tag="st",
                                            name=f"st{cq}_{p}_{t}")
                            tsl = slice(t * P, (t + 1) * P)
                            for hh in range(2):
                                nc.tensor.matmul(
                                    st[:, hh, :],
                                    lhsT=kt_t[p][hh * D : (hh + 1) * D, tsl].bitcast(F32R),
                                    rhs=qt_t[p][hh * D : (hh + 1) * D, qsl].bitcast(F32R),
                                    start=True,
                                    stop=True,
                                )
                            if t >= 4 * cq:
                                k = t - 4 * cq
                                nc.vector.tensor_add(st[:], st[:], masks[k][:])
                            pt = pt_pool.tile([P, 2, CH], F32R, tag="pt",
                                              name=f"pt{cq}_{p}_{t}")
                            nc.scalar.activation(pt[:], st[:], EXP)
                            for hh, ot in ((0, ot0), (1, ot1)):
                                h = 2 * p + hh
                                nc.tensor.matmul(
                                    ot[:],
                                    lhsT=vaug_t[t][:, h * (D + 1) : (h + 1) * (D + 1)].bitcast(F32R),
                                    rhs=pt[:, hh, :].bitcast(F32R),
                                    start=(t == 0),
                                    stop=(t == nts - 1),
                                )
                        for hh, ot in ((0, ot0), (1, ot1)):
                            recip = small_pool.tile([1, CH], F32R, tag="recip",
                                                    name=f"rc{cq}_{p}_{hh}")
                            with nc.allow_low_precision("fp32r denominators"):
                                nc.vector.reciprocal(recip[:], ot[D : D + 1, :])
                            bc = ps_st.tile([D, CH], F32, tag="st",
                                            name=f"bc{cq}_{p}_{hh}")
                            nc.tensor.matmul(
                                bc[:],
                                lhsT=ones_col[:].bitcast(F32R),
                                rhs=recip[:].bitcast(F32R),
                                start=True,
                                stop=True,
                            )
                            bcs = small_pool.tile([D, CH], F32, tag="bcs",
                                                  name=f"bcs{cq}_{p}_{hh}")
                            nc.vector.tensor_copy(bcs[:], bc[:])
                            nc.vector.tensor_mul(
                                ot_t[p][hh * D : (hh + 1) * D, qsl],
                                ot[0:D, :],
                                bcs[:],
                            )

            # ================= stage 3: out-projection =================
            with (
                tc.tile_pool(name="ysb", bufs=3) as y_pool,
                tc.tile_pool(name="psy", bufs=2, space="PSUM") as ps_y,
            ):
                for tt in range(NTT):
                    tsl = slice(tt * P, (tt + 1) * P)
                    for nn in range(2):
                        nsl = slice(nn * CH, (nn + 1) * CH)
                        yp = ps_y.tile([P, CH], F32, tag="y", name=f"y{tt}_{nn}")
                        for k in range(2):
                            nc.tensor.matmul(
                                yp[:],
                                lhsT=ot_t[k][:, tsl].bitcast(F32R),
                                rhs=wot_t[k][:, nsl].bitcast(F32R),
                                start=(k == 0),
                                stop=(k == 1),
                            )
                        ysb = y_pool.tile([P, CH], F32, tag="ysb", name=f"ysb{tt}_{nn}")
                        nc.vector.tensor_copy(ysb[:], yp[:])
                        nc.sync.dma_start(y[tsl, nsl], ysb[:])

    nc.finalize()
    return nc


_NC_CACHE = None


def get_nc():
    global _NC_CACHE
    if _NC_CACHE is None:
        _NC_CACHE = build_nc()
    return _NC_CACHE


def make_in_maps(x, Wq, Wk, Wv, Wo):
    scale = 1.0 / np.sqrt(np.float32(C))
    in_maps = []
    for core in range(8):
        b, hg = core // 4, core % 4
        hsl = slice(hg * NHPC, (hg + 1) * NHPC)
        xT = np.ascontiguousarray(x[b].T)
        wq = (Wq[hsl] * scale).transpose(1, 0, 2).reshape(C, DH)
        wk = Wk[hsl].transpose(1, 0, 2).reshape(C, DH)
        wv = Wv[hsl].transpose(1, 0, 2).reshape(C, DH)
        wqkv = np.ascontiguousarray(
            np.concatenate([wq, wk, wv], axis=1, dtype=np.float32))
        wot = np.ascontiguousarray(Wo[:, hg * DH : (hg + 1) * DH].T)
        in_maps.append({
            "xT": xT.astype(np.float32, copy=False),
            "Wqkv": wqkv,
            "WoT": wot.astype(np.float32, copy=False),
        })
    return in_maps


def gather(results, bo):
    out = np.zeros((B, T, C), dtype=np.float32)
    for core in range(8):
        out[core // 4] += results[core]["Y"]
    out += bo.astype(np.float32)
    return out


def kernel(x, Wq, Wk, Wv, Wo, bo, **run_kwargs):
    x = np.asarray(x, dtype=np.float32)
    Wq = np.asarray(Wq, dtype=np.float32)
    Wk = np.asarray(Wk, dtype=np.float32)
    Wv = np.asarray(Wv, dtype=np.float32)
    Wo = np.asarray(Wo, dtype=np.float32)
    bo = np.asarray(bo, dtype=np.float32)
    nc = get_nc()
    in_maps = make_in_maps(x, Wq, Wk, Wv, Wo)
    res = run_bass_kernel_spmd(nc, in_maps, core_ids=list(range(8)), **run_kwargs)
    out = gather(res.results, bo)
    if run_kwargs:
        return out, res
    return out
